# revision 36
# baseline (speedup 1.0000x reference)
"""Trainium2 Bass kernel for block-local (sliding-window) GQA attention with RoPE.

Module: x:[1,4096,2048] -> Q/K/V proj -> RoPE -> block-local attention
(window W=1024, block 1024, GQA 16 q-heads / 4 kv-heads, D=128) -> out proj.

Sharding: sequence-parallel over 8 cores, 512 queries per core. Each core
recomputes K/V for its 1536-row key span (queries + 1024 history, zero-padded
at the left edge), runs attention for all 16 heads on its query chunk, and
applies the full output projection locally; outputs concatenate over T.

Layout: feature-major ("transposed") activations. All matmuls run in
float32r at moving-dim >= 256 (full PE rate). Attention processes a full
GQA group (4 q-heads sharing one kv head) per matmul: Q is laid out
per-group [D, 4*128q] so score/PV/denominator matmuls stream 512 moving
columns per LDWEIGHTS, keeping the PE matmul-bound rather than
weight-load-bound. Softmax uses exp without max-subtraction (logits are
O(10)), masks via additive [128,512] triangle constants, denominators via
an all-ones stationary matmul interleaved with PV.
"""
import os
import sys

for _p in ("/root/.axon_site", "/root/.axon_site/_ro/trn_rl_repo", "/opt/trn_rl_repo"):
    if os.path.isdir(_p) and _p not in sys.path:
        sys.path.append(_p)

import numpy as np

import concourse.bass as bass
import concourse.tile as tile
import concourse.mybir as mybir
from concourse.vector_clock import ScopedClock
from concourse.bass_utils import run_bass_kernel_spmd

dt = mybir.dt

B, T, C = 1, 4096, 2048
H, HK, D = 16, 4, 128
W = 1024
THETA = 10000.0
NCORES = 8
TQ = T // NCORES            # 512 queries per core
TK = TQ + W                 # 1536-key span per core
NQC = TQ // 128             # 4 query chunks of 128
NJ = NQC + W // 128 - 3     # 9 key chunks per query chunk
NCT = C // 128              # 16 contraction tiles
NG = HK                     # 4 GQA groups of 4 q-heads
SCALE = 1.0 / float(np.sqrt(D))
NEG = -1.0e30


def _patch_tile_drain():
    """CoreV3 codegen caps sync-waits per instruction; the stock TileContext
    tail drain carries one wait per live semaphore.  Spill the waits across
    preceding sync-engine no-ops, one wait each."""
    if getattr(tile.TileContext, "_drain_patched", False):
        return

    def _drain_and_barrier(self, tick_clock, wait_clock):
        nc = self.nc
        probe = nc.sync.nop()
        wait_clock.add_sem_waits(
            probe.ins, ScopedClock({None: tick_clock.global_clock})
        )
        si = probe.ins.sync_info
        waits = list(si.on_wait) if si is not None and si.on_wait else []
        if len(waits) > 1:
            si.on_wait = waits[:1]
            for w in waits[1:]:
                extra = nc.sync.nop()
                extra.ins.sync_info = mybir.SyncInfo(on_wait=[w], on_update=[])
        nc.sync.drain()
        nc.all_engine_barrier()
        assert self.sems is not None
        popped = nc._tile_sem_poison_stack.pop()
        assert popped is self._sem_poison
        nc.clear_and_free_semaphores(list(self.sems.allocated().values()))
        nc.all_engine_barrier()

    tile.TileContext._drain_and_barrier = _drain_and_barrier
    tile.TileContext._drain_patched = True


_MAX_WAITS = 1


def _spill_excess_waits(nc):
    """Walrus codegen caps sync-waits per instruction.  For any instruction
    carrying more, move the excess onto same-engine no-ops inserted just
    before it (engines execute in program order, so the waits still resolve
    before the instruction runs)."""
    n = [0]
    for f in nc.m.functions:
        for bb in f.blocks:
            out = []
            for inst in bb.instructions:
                si = inst.sync_info
                waits = list(si.on_wait) if si is not None and si.on_wait else []
                if len(waits) > _MAX_WAITS:
                    for lo in range(0, len(waits) - _MAX_WAITS, _MAX_WAITS):
                        nop = mybir.InstNoOp(
                            name=f"waitspill-{n[0]}", ins=[], outs=[]
                        )
                        n[0] += 1
                        nop.engine = inst.engine
                        nop.sync_info = mybir.SyncInfo(
                            on_wait=waits[lo:lo + _MAX_WAITS], on_update=[]
                        )
                        out.append(nop)
                    si.on_wait = waits[len(waits) - _MAX_WAITS:]
                out.append(inst)
            bb.instructions[:] = out


def _rope_tables():
    d2 = np.arange(0, D, 2, dtype=np.float64) / D
    ts = THETA ** d2
    ang = np.arange(T, dtype=np.float64)[:, None] / ts[None, :]
    ang = np.concatenate([ang, ang], axis=1)            # [T, D]
    cosT = np.cos(ang).T                                # [D, T]
    sinS = np.sin(ang).T
    sinS[: D // 2] *= -1.0    # rot(u)[d<64] = -u[d+64]; out = u*cos + shift(u)*sinS
    return cosT.astype(np.float32), sinS.astype(np.float32)


def _build_program():
    nc = bass.Bass(num_swdge_queues=4)
    f32, f32r = dt.float32, dt.float32r

    xh_e = nc.declare_dram_parameter("xh", [C, W], f32r, isOutput=False)
    xq_e = nc.declare_dram_parameter("xq", [C, TQ], f32r, isOutput=False)
    wq_e = nc.declare_dram_parameter("wq", [C, H * D], f32r, isOutput=False)
    wk_e = nc.declare_dram_parameter("wk", [C, HK * D], f32r, isOutput=False)
    wv_e = nc.declare_dram_parameter("wv", [C, HK * D], f32r, isOutput=False)
    wo_e = nc.declare_dram_parameter("wo", [H * D, C], f32r, isOutput=False)
    cos_e = nc.declare_dram_parameter("cosk", [D, TK], f32, isOutput=False)
    sin_e = nc.declare_dram_parameter("sink", [D, TK], f32, isOutput=False)
    triw_e = nc.declare_dram_parameter("triw", [128, 512], f32, isOutput=False)
    tric_e = nc.declare_dram_parameter("tric", [128, 512], f32, isOutput=False)
    pad_e = nc.declare_dram_parameter("padfix", [128, NQC * 512], f32, isOutput=False)
    ones_e = nc.declare_dram_parameter("onesc", [128, 128], f32r, isOutput=False)
    y_e = nc.declare_dram_parameter("y", [TQ, C], f32, isOutput=True)

    Exp = mybir.ActivationFunctionType.Exp

    with tile.TileContext(nc) as tc:
        with (
            tc.tile_pool(name="vout", bufs=1) as vout,
            tc.tile_pool(name="krp", bufs=1) as krp,
        ):
            # Pool enter order is the (per-side) free stack order: wvp is
            # freed first, then xhp, then ropep, then xqp.  DMA emission
            # order is priority: xq + wv feed the first matmuls (V
            # projection of the query span).
            xqp_cm = tc.tile_pool(name="xqp", bufs=1)
            xqp = xqp_cm.__enter__()
            ropep_cm = tc.tile_pool(name="ropep", bufs=1)
            ropep = ropep_cm.__enter__()
            xhp_cm = tc.tile_pool(name="xhp", bufs=1)
            xhp = xhp_cm.__enter__()
            wvp_cm = tc.tile_pool(name="wvp", bufs=1)
            wvp = wvp_cm.__enter__()

            # interleave xq/wv so the first V matmul (accumulating over ct)
            # can start as soon as the first (xq, wv) tile pair lands
            xq_t = [xqp.tile([128, TQ], f32r, name=f"xq{ct}", tag=f"xq{ct}")
                    for ct in range(NCT)]
            wv_t = [wvp.tile([128, HK * D], f32r, name=f"wv{ct}", tag=f"wv{ct}")
                    for ct in range(NCT)]
            for ct in range(NCT):
                nc.gpsimd.dma_start(xq_t[ct][:], xq_e[128 * ct:128 * (ct + 1), :])
                nc.gpsimd.dma_start(wv_t[ct][:], wv_e[128 * ct:128 * (ct + 1), :])

            xh_t = [xhp.tile([128, W], f32r, name=f"xh{ct}", tag=f"xh{ct}")
                    for ct in range(NCT)]
            for ct in range(NCT):
                nc.gpsimd.dma_start(xh_t[ct][:], xh_e[128 * ct:128 * (ct + 1), :])

            cosk = ropep.tile([D, TK], f32, tag="cosk")
            sink = ropep.tile([D, TK], f32, tag="sink")

            v_t = [vout.tile([128, HK * D], f32r, name=f"v{tt}", tag=f"v{tt}")
                   for tt in range(TK // 128)]
            kr = [krp.tile([D, TK], f32r, name=f"kr{g}", tag=f"kr{g}")
                  for g in range(HK)]
            an = {}

            # ---------------- projection phases (own PSUM pool) ----------
            pp_cm = tc.tile_pool(name="pp", bufs=4, space="PSUM")
            pp = pp_cm.__enter__()

            def xk_slice(ct, lo, size):
                # local key cols [lo, lo+size) from history (0..W) / query (W..TK)
                if lo + size <= W:
                    return xh_t[ct][:, lo:lo + size]
                assert lo >= W
                return xq_t[ct][:, lo - W:lo - W + size]

            # ---- V projection: v[t, hd] = x[t, :] @ wv ------------------
            # query-span tiles first: they only need xq, which loads first.
            for tt in list(range(W // 128, TK // 128)) + list(range(W // 128)):
                ps = pp.tile([128, HK * D], dt.float32, tag="pp")
                for ct in range(NCT):
                    nc.tensor.matmul(
                        ps[:], xk_slice(ct, 128 * tt, 128), wv_t[ct][:],
                        start=(ct == 0), stop=(ct == NCT - 1),
                    )
                nc.scalar.copy(v_t[tt][:], ps[:])
            wvp_cm.__exit__(None, None, None)

            # ---- K projection + RoPE: krT[d, t] -------------------------
            with (
                tc.tile_pool(name="wkm", bufs=2) as wkm,
                tc.tile_pool(name="shf", bufs=2) as shf,
            ):
                for g in range(HK):
                    wslab = wkm.tile([128, C], f32r, tag="wkm")
                    src = wk_e[:, 128 * g:128 * (g + 1)].rearrange(
                        "(a p) m -> p a m", p=128
                    )
                    nc.gpsimd.dma_start(
                        wslab[:].rearrange("p (a m) -> p a m", a=NCT), src
                    )
                    if g == 0:
                        # queue the rope tables right behind the first wk slab
                        nc.gpsimd.dma_start(cosk[:], cos_e[:])
                        nc.gpsimd.dma_start(sink[:], sin_e[:])
                    for tcb in (2, 0, 1):
                        ps = pp.tile([128, 512], dt.float32, tag="pp")
                        for ct in range(NCT):
                            nc.tensor.matmul(
                                ps[:], wslab[:, 128 * ct:128 * (ct + 1)],
                                xk_slice(ct, 512 * tcb, 512),
                                start=(ct == 0), stop=(ct == NCT - 1),
                            )
                        sl = slice(512 * tcb, 512 * (tcb + 1))
                        qs = shf.tile([128, 512], dt.float32, tag="qs")
                        nc.scalar.copy(qs[0:64, :], ps[64:128, :])
                        nc.scalar.copy(qs[64:128, :], ps[0:64, :])
                        nc.vector.tensor_mul(kr[g][:, sl], ps[:], cosk[:, sl])
                        nc.vector.tensor_mul(qs[:], qs[:], sink[:, sl])
                        nc.vector.tensor_add(kr[g][:, sl], kr[g][:, sl], qs[:])

            xhp_cm.__exit__(None, None, None)
            pp_cm.__exit__(None, None, None)
            pp2_cm = tc.tile_pool(name="pp2", bufs=3, space="PSUM", side="right")
            pp2 = pp2_cm.__enter__()

            # ---- Q projection + RoPE (query columns only) ---------------
            # qg[g] layout: [128, 2048] = 4 qc-blocks of 512 = 4 heads x 128q
            qpr_cm = tc.tile_pool(name="qpr", bufs=1, side="right")
            qpr = qpr_cm.__enter__()
            qg = [qpr.tile([D, NQC * 512], f32r, name=f"qg{g}", tag=f"qg{g}")
                  for g in range(NG)]
            with (
                tc.tile_pool(name="wqm", bufs=3) as wqm,
                tc.tile_pool(name="shq", bufs=2) as shq,
            ):
                for m in range(H):
                    g, h4 = m // 4, m % 4
                    wslab = wqm.tile([128, C], f32r, tag="wqm")
                    src = wq_e[:, 128 * m:128 * (m + 1)].rearrange(
                        "(a p) m -> p a m", p=128
                    )
                    # HWDGE (SP) ring: the 16.8MB wq stream drains in
                    # parallel with the SWDGE queues instead of behind them
                    # (SP is idle until the late y stores; ACT would stall
                    # these behind sem-waiting V copies)
                    nc.sync.dma_start(
                        wslab[:].rearrange("p (a m) -> p a m", a=NCT), src
                    )
                    ps = pp2.tile([128, TQ], dt.float32, tag="pp2")
                    for ct in range(NCT):
                        nc.tensor.matmul(
                            ps[:], wslab[:, 128 * ct:128 * (ct + 1)], xq_t[ct][:],
                            start=(ct == 0), stop=(ct == NCT - 1),
                        )
                    qs = shq.tile([128, TQ], dt.float32, tag="qs")
                    nc.scalar.copy(qs[0:64, :], ps[64:128, :])
                    nc.scalar.copy(qs[64:128, :], ps[0:64, :])
                    csl = slice(W, W + TQ)
                    nc.vector.tensor_mul(qs[:], qs[:], sink[:, csl])
                    for qc in range(NQC):
                        dsl = slice(512 * qc + 128 * h4, 512 * qc + 128 * h4 + 128)
                        ssl = slice(128 * qc, 128 * (qc + 1))
                        nc.vector.tensor_mul(
                            qg[g][:, dsl], ps[:, ssl],
                            cosk[:, W + 128 * qc:W + 128 * (qc + 1)]
                        )
                        nc.vector.tensor_add(qg[g][:, dsl], qg[g][:, dsl], qs[:, ssl])

            ropep_cm.__exit__(None, None, None)
            xqp_cm.__exit__(None, None, None)
            pp2_cm.__exit__(None, None, None)

            # attention-phase constants (queued behind the wq slabs)
            cst2_cm = tc.tile_pool(name="cst2", bufs=1, side="right")
            cst2 = cst2_cm.__enter__()
            triw = cst2.tile([128, 512], f32, tag="triw")
            tric = cst2.tile([128, 512], f32, tag="tric")
            padf = cst2.tile([128, NQC * 512], f32, tag="padf")
            ones = cst2.tile([128, 128], f32r, tag="ones")
            nc.gpsimd.dma_start(triw[:], triw_e[:])
            nc.gpsimd.dma_start(tric[:], tric_e[:])
            nc.gpsimd.dma_start(padf[:], pad_e[:])
            nc.gpsimd.dma_start(ones[:], ones_e[:])

            # ---- prefetch first out-projection weight chunks ------------
            attnp_cm = tc.tile_pool(name="attn", bufs=1, side="right")
            attnp = attnp_cm.__enter__()
            wop_cm = tc.tile_pool(name="wop", bufs=24, side="right")
            wop = wop_cm.__enter__()
            wo_t = {}

            def load_wo(cc, eng=None):
                for h in range(H):
                    wt = wop.tile([128, 512], f32r, name=f"wo{h}_{cc}", tag="wo")
                    (eng or nc.sync).dma_start(
                        wt[:], wo_e[128 * h:128 * (h + 1), 512 * cc:512 * (cc + 1)]
                    )
                    wo_t[(h, cc)] = wt

            load_wo(0)

            # ---- attention ----------------------------------------------
            sc_cm = tc.tile_pool(name="sc", bufs=4, space="PSUM")
            psc = sc_cm.__enter__()
            at_cm = tc.tile_pool(name="at", bufs=2, space="PSUM")
            pat = at_cm.__enter__()
            su_cm = tc.tile_pool(name="su", bufs=2, space="PSUM")
            psu = su_cm.__enter__()
            pb_cm = tc.tile_pool(name="pb", bufs=6)
            pbp = pb_cm.__enter__()
            rc_cm = tc.tile_pool(name="rc", bufs=3)
            rcp = rc_cm.__enter__()

            PAIRS = [(0, 1), (2, 3), (4, 5), (6, 7), (8,)]
            for qc in range(NQC):
                for g in range(NG):
                    at_ps = pat.tile([128, 512], dt.float32, tag="at")
                    su_ps = psu.tile([128, 512], dt.float32, tag="su")
                    qsl = slice(512 * qc, 512 * (qc + 1))

                    def emit_sc(pair):
                        pbs = []
                        for j in pair:
                            sc_ps = psc.tile([128, 512], dt.float32, tag="sc")
                            lk = 128 * (qc + j)
                            nc.tensor.matmul(
                                sc_ps[:], kr[g][:, lk:lk + 128], qg[g][:, qsl],
                                start=True, stop=True, skip_group_check=True,
                            )
                            if j == 0:
                                nc.vector.tensor_add(sc_ps[:], sc_ps[:], triw[:])
                            if j == NJ - 1:
                                nc.vector.tensor_add(sc_ps[:], sc_ps[:], tric[:])
                            pb = pbp.tile([128, 512], f32r, tag="pb")
                            nc.scalar.activation(pb[:], sc_ps[:], Exp, scale=SCALE)
                            pbs.append((j, pb))
                        return pbs

                    def emit_pv(pbs):
                        for j, pb in pbs:
                            nc.tensor.matmul(
                                at_ps[:], v_t[qc + j][:, 128 * g:128 * (g + 1)],
                                pb[:], start=(j == 0), stop=(j == NJ - 1),
                                skip_group_check=True,
                            )
                            nc.tensor.matmul(
                                su_ps[:], ones[:], pb[:],
                                start=(j == 0), stop=(j == NJ - 1),
                                skip_group_check=True,
                            )

                    pending = emit_sc(PAIRS[0])
                    for p in range(len(PAIRS)):
                        nxt = emit_sc(PAIRS[p + 1]) if p + 1 < len(PAIRS) else None
                        emit_pv(pending)
                        pending = nxt

                    rec = rcp.tile([128, 512], dt.float32, tag="rc")
                    nc.vector.tensor_sub(rec[:], su_ps[:], padf[:, 512 * qc:512 * (qc + 1)])
                    nc.vector.reciprocal(rec[:], rec[:])
                    a = attnp.tile([128, 512], f32r, name=f"an{g}_{qc}",
                                   tag=f"an{g}_{qc}")
                    an[(g, qc)] = a
                    nc.vector.tensor_mul(a[:], at_ps[:], rec[:])
                if qc == 0:
                    load_wo(1, eng=nc.gpsimd)

            rc_cm.__exit__(None, None, None)
            pb_cm.__exit__(None, None, None)
            su_cm.__exit__(None, None, None)
            at_cm.__exit__(None, None, None)
            sc_cm.__exit__(None, None, None)

            # ---- output projection --------------------------------------
            with (
                tc.tile_pool(name="yp", bufs=2, space="PSUM") as pyp,
                tc.tile_pool(name="ych", bufs=4) as ychp,
            ):
                for cc in range(4):
                    if cc + 2 < 4 and (cc + 2) not in (0, 1):
                        load_wo(cc + 2, eng=nc.gpsimd)
                    for qc in range(NQC):
                        ps = pyp.tile([128, 512], dt.float32, tag="yp")
                        for m in range(H):
                            g, h4 = m // 4, m % 4
                            nc.tensor.matmul(
                                ps[:], an[(g, qc)][:, 128 * h4:128 * (h4 + 1)],
                                wo_t[(m, cc)][:],
                                start=(m == 0), stop=(m == H - 1),
                            )
                        ych = ychp.tile([128, 512], dt.float32, tag="ych")
                        nc.scalar.copy(ych[:], ps[:])
                        # HWDGE path: y stores bypass the gpsimd queues so they
                        # don't sit behind the remaining wo prefetches
                        nc.sync.dma_start(
                            y_e[128 * qc:128 * (qc + 1), 512 * cc:512 * (cc + 1)], ych[:]
                        )
            wop_cm.__exit__(None, None, None)
            attnp_cm.__exit__(None, None, None)
            cst2_cm.__exit__(None, None, None)
            qpr_cm.__exit__(None, None, None)
    _spill_excess_waits(nc)
    return nc


def _host_inputs(x, q_kernel, k_kernel, v_kernel, out_kernel):
    x2 = np.ascontiguousarray(np.asarray(x, np.float32)[0])      # [T, C]
    xT = np.zeros((C, W + T), np.float32)
    xT[:, W:] = x2.T
    cosT, sinS = _rope_tables()
    cos_pad = np.concatenate([np.repeat(cosT[:, :1], W, axis=1), cosT], axis=1)
    sin_pad = np.concatenate([np.repeat(sinS[:, :1], W, axis=1), sinS], axis=1)

    i1 = np.arange(128)
    tri1 = np.where(i1[None, :] <= i1[:, None], 0.0, NEG).astype(np.float32)
    tri2 = np.where(i1[None, :] >= i1[:, None], 0.0, NEG).astype(np.float32)
    triw = np.tile(tri1, (1, 4))   # valid qi <= kj, per head block
    tric = np.tile(tri2, (1, 4))   # valid qi >= kj

    wq = np.ascontiguousarray(np.asarray(q_kernel, np.float32))
    wk = np.ascontiguousarray(np.asarray(k_kernel, np.float32))
    wv = np.ascontiguousarray(np.asarray(v_kernel, np.float32))
    wo = np.ascontiguousarray(np.asarray(out_kernel, np.float32))

    in_maps = []
    for core in range(NCORES):
        q0 = TQ * core
        xk = xT[:, q0:q0 + TK]
        npad = max(0, (W - q0) // 128)
        padf = np.zeros((128, NQC * 512), np.float32)
        qi = np.arange(128, dtype=np.float32)
        for qc in range(NQC):
            pv = np.zeros(128, np.float32)
            if qc < npad:
                pv += 128.0 - qi           # j=0 window chunk: valid count #{kj >= qi}
            for j in range(1, NJ - 1):
                if qc + j < npad:
                    pv += 128.0
            blk = np.tile(pv, 4)           # same for all 4 heads of a group
            padf[:, 512 * qc:512 * (qc + 1)] = blk[None, :]
        in_maps.append({
            "xh": np.ascontiguousarray(xk[:, :W]),
            "xq": np.ascontiguousarray(xk[:, W:]),
            "wq": wq, "wk": wk, "wv": wv, "wo": wo,
            "cosk": np.ascontiguousarray(cos_pad[:, q0:q0 + TK]),
            "sink": np.ascontiguousarray(sin_pad[:, q0:q0 + TK]),
            "triw": triw, "tric": tric, "padfix": padf,
            "onesc": np.ones((128, 128), np.float32),
        })
    return in_maps


_CACHED = {}


def kernel(x, q_kernel, k_kernel, v_kernel, out_kernel, _profile=False):
    _patch_tile_drain()
    if "nc" not in _CACHED:
        _CACHED["nc"] = _build_program()
    nc = _CACHED["nc"]
    in_maps = _host_inputs(x, q_kernel, k_kernel, v_kernel, out_kernel)
    res = run_bass_kernel_spmd(nc, in_maps, list(range(NCORES)), trace=_profile)
    y = np.concatenate([res.results[i]["y"] for i in range(NCORES)], axis=0)
    out = y[None, :, :].astype(np.float32)
    if _profile:
        return out, res
    return out
